# revision 3
# baseline (speedup 1.0000x reference)
"""APPNP GNN kernel for 8 Trainium2 NeuronCores.

Sharding: nodes (dst side) split into 8 contiguous shards of 12500.
Per step: all-gather of the dinv-scaled feature table (25.6 MB), then each
core gathers per-edge source rows from its local copy via dma_gather and
segment-sums them by dst with one-hot matmuls accumulating in PSUM.
Norms are folded into per-node scales: norm(s,d) = dinv[s]*dinv[d], so the
gathered table is pre-scaled by dinv and the dst-side dinv folds into the
PSUM evacuation. Self-loops are handled analytically (no gather).
"""
import os
import sys

sys.path.insert(0, "/opt/trn_rl_repo")

import numpy as np

N = 100000
E = 3200000
IN = 256
HID = 256
OUT = 64
K = 3
ALPHA = 0.1
NCORES = 8
NLOC = N // NCORES          # 12500
NBLK = (NLOC + 127) // 128  # 98 dst blocks per core
RANGE = 32768               # int16 index range
NRANGES = (N + RANGE - 1) // RANGE  # 4
MAX_TILES_PER_CALL = 8      # 1024 idxs per dma_gather (single-packet limit)


def _pack_idx_call(idx):
    """[n] int array -> [128, n//16] int16 tile (i -> [i%16, i//16], x8 replicated)."""
    n = len(idx)
    t = idx.reshape(n // 16, 16).T.astype(np.int16)
    return np.tile(t, (8, 1))


def _preprocess(x, edge_index, W1, b1, W2, b2):
    src = np.asarray(edge_index[0], dtype=np.int64)
    dst = np.asarray(edge_index[1], dtype=np.int64)
    deg = np.bincount(dst, minlength=N).astype(np.float64) + 1.0  # +self loop
    dinv = (1.0 / np.sqrt(deg)).astype(np.float32)

    # per-core edge buckets
    core_of = dst // NLOC
    per_core = []
    for c in range(NCORES):
        m = core_of == c
        s_c = src[m]
        d_c = dst[m] - c * NLOC
        blk = d_c // 128
        drel = d_c % 128
        rng_ = s_c // RANGE
        order = np.lexsort((s_c, rng_, blk))
        per_core.append((s_c[order], drel[order], blk[order], rng_[order]))

    # bucket counts per (core, block, range)
    cnt = np.zeros((NCORES, NBLK, NRANGES), dtype=np.int64)
    for c in range(NCORES):
        s_c, drel, blk, rng_ = per_core[c]
        np.add.at(cnt[c], (blk, rng_), 1)
    tiles_common = np.maximum(np.ceil(cnt / 128.0).astype(np.int64).max(axis=0), 0)

    # shared call schedule: list of (block, range, ntiles_in_call)
    calls = []
    for b in range(NBLK):
        for r in range(NRANGES):
            t = int(tiles_common[b, r])
            while t > 0:
                ct = min(t, MAX_TILES_PER_CALL)
                calls.append((b, r, ct))
                t -= ct
    T_total = int(tiles_common.sum())

    # per-core streams in the common layout
    idx_streams, dstrel_streams = [], []
    for c in range(NCORES):
        s_c, drel, blk, rng_ = per_core[c]
        # bucket start offsets in the sorted arrays
        starts = np.zeros((NBLK, NRANGES), dtype=np.int64)
        flat = cnt[c].reshape(-1)
        starts.reshape(-1)[1:] = np.cumsum(flat)[:-1]
        idx_cols = []    # per call: [128, ct*8] int16
        drel_cols = []   # per tile: [128] float32
        for b, r, ct in calls:
            # consume up to ct*128 edges from this bucket (tracked via starts/cnt copy)
            pass
        # do a second pass with mutable cursors
        cur = starts.copy()
        remaining = cnt[c].copy()
        for b, r, ct in calls:
            n_take = min(int(remaining[b, r]), ct * 128)
            off = int(cur[b, r])
            e_idx = (s_c[off:off + n_take] - r * RANGE).astype(np.int64)
            e_drel = drel[off:off + n_take].astype(np.float32)
            cur[b, r] += n_take
            remaining[b, r] -= n_take
            pad = ct * 128 - n_take
            if pad:
                e_idx = np.concatenate([e_idx, np.zeros(pad, np.int64)])
                e_drel = np.concatenate([e_drel, np.full(pad, -1.0, np.float32)])
            idx_cols.append(_pack_idx_call(e_idx))
            for j in range(ct):
                drel_cols.append(e_drel[j * 128:(j + 1) * 128])
        idx_streams.append(np.concatenate(idx_cols, axis=1))          # [128, 8*T_total]
        dstrel_streams.append(np.stack(drel_cols, axis=1))            # [128, T_total]

    # per-core dense inputs
    in_maps = []
    for c in range(NCORES):
        lo, hi = c * NLOC, (c + 1) * NLOC
        xT = np.ascontiguousarray(x[lo:hi].T)            # [256, 12500]
        dv = dinv[lo:hi]
        dinv_col = np.zeros((128, NBLK), np.float32)
        for b in range(NBLK):
            nblk = min(128, NLOC - b * 128)
            dinv_col[:nblk, b] = dv[b * 128:b * 128 + nblk]
        W1p = np.zeros((128, 512), np.float32)           # [k, (k*2+m) blocks]
        for k in range(2):
            for mblk in range(2):
                W1p[:, (k * 2 + mblk) * 128:(k * 2 + mblk + 1) * 128] = \
                    W1[k * 128:(k + 1) * 128, mblk * 128:(mblk + 1) * 128]
        W2p = np.zeros((128, 128), np.float32)           # [hid, m*64+o]
        for mblk in range(2):
            W2p[:, mblk * 64:(mblk + 1) * 64] = W2[mblk * 128:(mblk + 1) * 128, :]
        b1c = np.stack([b1[:128], b1[128:]], axis=1).astype(np.float32)   # [128, 2]
        b2r = np.tile(b2[None, :], (128, 1)).astype(np.float32)           # [128, 64]
        in_maps.append({
            "xT": xT.astype(np.float32),
            "idxs": idx_streams[c],
            "dstrel": dstrel_streams[c],
            "dinvc": dinv_col,
            "W1p": W1p, "W2p": W2p, "b1c": b1c, "b2r": b2r,
        })
    return in_maps, calls, T_total


def _build(calls, T_total):
    from concourse import bass, bacc, mybir
    from concourse.tile import TileContext

    DT = mybir.dt.float32
    nc = bacc.Bacc("TRN2", target_bir_lowering=False, debug=False,
                   num_devices=NCORES)

    xT_d = nc.dram_tensor("xT", [IN, NLOC], DT, kind="ExternalInput").ap()
    idxs_d = nc.dram_tensor("idxs", [128, 8 * T_total], mybir.dt.int16,
                            kind="ExternalInput").ap()
    dstrel_d = nc.dram_tensor("dstrel", [128, T_total], DT,
                              kind="ExternalInput").ap()
    dinvc_d = nc.dram_tensor("dinvc", [128, NBLK], DT, kind="ExternalInput").ap()
    W1p_d = nc.dram_tensor("W1p", [128, 512], DT, kind="ExternalInput").ap()
    W2p_d = nc.dram_tensor("W2p", [128, 128], DT, kind="ExternalInput").ap()
    b1c_d = nc.dram_tensor("b1c", [128, 2], DT, kind="ExternalInput").ap()
    b2r_d = nc.dram_tensor("b2r", [128, 64], DT, kind="ExternalInput").ap()
    out_d = nc.dram_tensor("out", [NLOC, OUT], DT, kind="ExternalOutput").ap()

    outp_loc = nc.dram_tensor("outp_loc", [NLOC, OUT], DT).ap()
    tables = [nc.dram_tensor(f"table{k}", [N, OUT], DT, addr_space="Shared").ap()
              for k in range(K)]

    from concourse.masks import make_identity

    with TileContext(nc) as tc:
        with tc.tile_pool(name="const", bufs=1) as constp, \
             tc.tile_pool(name="persist", bufs=1) as persist:
            ident = constp.tile([128, 128], DT)
            make_identity(nc, ident[:])
            iota_f = constp.tile([128, MAX_TILES_PER_CALL * 128], DT)
            iota_i = constp.tile([128, MAX_TILES_PER_CALL * 128], mybir.dt.int32)
            nc.gpsimd.iota(iota_i[:], pattern=[[0, MAX_TILES_PER_CALL], [1, 128]],
                           base=0, channel_multiplier=0)
            nc.vector.tensor_copy(iota_f[:], iota_i[:])
            dinv_sb = constp.tile([128, NBLK], DT)
            nc.sync.dma_start(out=dinv_sb[:], in_=dinvc_d[:])
            A_sb = constp.tile([128, NBLK], DT)      # 0.9*dinv^2
            nc.vector.tensor_tensor(out=A_sb[:], in0=dinv_sb[:], in1=dinv_sb[:],
                                    op=mybir.AluOpType.mult)
            nc.vector.tensor_scalar_mul(A_sb[:], A_sb[:], 1.0 - ALPHA)
            C_sb = constp.tile([128, NBLK], DT)      # 0.9*dinv
            nc.vector.tensor_scalar_mul(C_sb[:], dinv_sb[:], 1.0 - ALPHA)
            dstrel_sb = persist.tile([128, T_total], DT)
            nc.sync.dma_start(out=dstrel_sb[:], in_=dstrel_d[:])
            B_sb = persist.tile([128, NBLK * OUT], DT)   # 0.1*dinv*h
            D_sb = persist.tile([128, NBLK * OUT], DT)   # 0.1*h
            outp_sb = persist.tile([128, NBLK * OUT], DT)  # dinv*out_prev (local)

            # ---------------- MLP ----------------
            with tc.tile_pool(name="mlpw", bufs=1) as mlpw, \
                 tc.tile_pool(name="mlp", bufs=3) as mlp, \
                 tc.tile_pool(name="mpsum", bufs=2, space="PSUM") as mpsum, \
                 tc.tile_pool(name="mpsum2", bufs=2, space="PSUM") as mpsum2:
                W1_sb = mlpw.tile([128, 512], DT)
                nc.sync.dma_start(out=W1_sb[:], in_=W1p_d[:])
                W2_sb = mlpw.tile([128, 128], DT)
                nc.sync.dma_start(out=W2_sb[:], in_=W2p_d[:])
                b1_sb = mlpw.tile([128, 2], DT)
                nc.sync.dma_start(out=b1_sb[:], in_=b1c_d[:])
                b2_sb = mlpw.tile([128, 64], DT)
                nc.sync.dma_start(out=b2_sb[:], in_=b2r_d[:])

                for rb in range(NBLK):
                    r0 = rb * 128
                    nrow = min(128, NLOC - r0)
                    xt = mlp.tile([128, 2 * 128], DT, tag="xt")
                    for k in range(2):
                        nc.sync.dma_start(
                            out=xt[:, k * 128:k * 128 + nrow],
                            in_=xT_d[k * 128:(k + 1) * 128, r0:r0 + nrow])
                    h1 = mlp.tile([128, 2 * 128], DT, tag="h1")
                    for mblk in range(2):
                        p1 = mpsum.tile([128, 128], DT, tag="p1")
                        for k in range(2):
                            nc.tensor.matmul(
                                out=p1[:, :nrow],
                                lhsT=W1_sb[:, (k * 2 + mblk) * 128:(k * 2 + mblk + 1) * 128],
                                rhs=xt[:, k * 128:k * 128 + nrow],
                                start=(k == 0), stop=(k == 1))
                        nc.scalar.activation(
                            h1[:, mblk * 128:mblk * 128 + nrow], p1[:, :nrow],
                            mybir.ActivationFunctionType.Relu,
                            bias=b1_sb[:, mblk:mblk + 1])
                    p2 = mpsum2.tile([128, 64], DT, tag="p2")
                    for mblk in range(2):
                        nc.tensor.matmul(
                            out=p2[:nrow, :],
                            lhsT=h1[:, mblk * 128:mblk * 128 + nrow],
                            rhs=W2_sb[:, mblk * 64:(mblk + 1) * 64],
                            start=(mblk == 0), stop=(mblk == 1))
                    ht = mlp.tile([128, 64], DT, tag="ht")
                    nc.vector.tensor_tensor(out=ht[:nrow], in0=p2[:nrow],
                                            in1=b2_sb[:nrow],
                                            op=mybir.AluOpType.add)
                    ob = slice(rb * OUT, rb * OUT + OUT)
                    # outp_sb = dinv*h ; B = 0.1*dinv*h ; D = 0.1*h
                    nc.vector.tensor_scalar(
                        out=outp_sb[:nrow, ob], in0=ht[:nrow],
                        scalar1=dinv_sb[:nrow, rb:rb + 1], scalar2=None,
                        op0=mybir.AluOpType.mult)
                    nc.vector.tensor_scalar_mul(B_sb[:nrow, ob],
                                                outp_sb[:nrow, ob], ALPHA)
                    nc.vector.tensor_scalar_mul(D_sb[:nrow, ob], ht[:nrow], ALPHA)
                    nc.sync.dma_start(out=outp_loc[r0:r0 + nrow, :],
                                      in_=outp_sb[:nrow, ob])

            # ---------------- propagation ----------------
            range_lens = [min(RANGE, N - r * RANGE) for r in range(NRANGES)]
            for step in range(K):
                table = tables[step]
                nc.gpsimd.collective_compute(
                    "AllGather", mybir.AluOpType.bypass,
                    replica_groups=[list(range(NCORES))],
                    ins=[outp_loc[:].opt()],
                    outs=[table[:].opt()])
                with tc.tile_pool(name=f"gat{step}", bufs=4) as gat, \
                     tc.tile_pool(name=f"idx{step}", bufs=4) as idxp, \
                     tc.tile_pool(name=f"sbl{step}", bufs=4) as sbl, \
                     tc.tile_pool(name=f"agg{step}", bufs=2) as aggp, \
                     tc.tile_pool(name=f"ev{step}", bufs=3) as evp, \
                     tc.tile_pool(name=f"ps{step}", bufs=2, space="PSUM") as psp, \
                     tc.tile_pool(name=f"pt{step}", bufs=2, space="PSUM") as ptp:
                    tile_cursor = 0
                    call_cursor = 0
                    cur_block = -1
                    aggT_ps = None
                    # group calls by block
                    from itertools import groupby
                    ci = 0
                    for b, bcalls_iter in groupby(calls, key=lambda x: x[0]):
                        bcalls = list(bcalls_iter)
                        ntiles_b = sum(ct for (_, _, ct) in bcalls)
                        nd = min(128, NLOC - b * 128)
                        aggT_ps = psp.tile([64, 128], DT, tag="aggT")
                        t_in_b = 0
                        for (_, r, ct) in bcalls:
                            it = idxp.tile([128, MAX_TILES_PER_CALL * 8],
                                           mybir.dt.int16, tag="it")
                            nc.sync.dma_start(
                                out=it[:, :ct * 8],
                                in_=idxs_d[:, tile_cursor * 8:(tile_cursor + ct) * 8])
                            gt = gat.tile([128, MAX_TILES_PER_CALL, OUT], DT, tag="gt")
                            nc.gpsimd.dma_gather(
                                gt[:, :ct, :],
                                table[r * RANGE:r * RANGE + range_lens[r]],
                                it[:, :ct * 8],
                                ct * 128, ct * 128, OUT,
                                single_packet=True)
                            st = sbl.tile([128, MAX_TILES_PER_CALL * 128], DT, tag="st")
                            nc.vector.tensor_tensor(
                                out=st[:, :ct * 128].rearrange(
                                    "p (a b) -> p a b", b=128),
                                in0=dstrel_sb[:, tile_cursor:tile_cursor + ct]
                                    .unsqueeze(2).to_broadcast([128, ct, 128]),
                                in1=iota_f[:, :ct * 128].rearrange(
                                    "p (a b) -> p a b", b=128),
                                op=mybir.AluOpType.is_equal)
                            for j in range(ct):
                                nc.tensor.matmul(
                                    out=aggT_ps[:, :],
                                    lhsT=gt[:, j, :],
                                    rhs=st[:, j * 128:(j + 1) * 128],
                                    start=(t_in_b == 0),
                                    stop=(t_in_b == ntiles_b - 1))
                                t_in_b += 1
                            tile_cursor += ct
                        # evacuate block b
                        aggT_sb = evp.tile([64, 128], DT, tag="aggTs")
                        nc.scalar.mul(aggT_sb[:], aggT_ps[:], 1.0)
                        tr_ps = ptp.tile([128, 64], DT, tag="tr")
                        nc.tensor.transpose(out=tr_ps[:, :], in_=aggT_sb[:, :],
                                            identity=ident[:64, :64])
                        ob = slice(b * OUT, b * OUT + OUT)
                        tmp = evp.tile([128, 64], DT, tag="tmp")
                        nc.vector.tensor_tensor(out=tmp[:nd], in0=tr_ps[:nd],
                                                in1=outp_sb[:nd, ob],
                                                op=mybir.AluOpType.add)
                        if step < K - 1:
                            nc.vector.tensor_scalar(
                                out=outp_sb[:nd, ob], in0=tmp[:nd],
                                scalar1=A_sb[:nd, b:b + 1], scalar2=None,
                                op0=mybir.AluOpType.mult)
                            nc.vector.tensor_tensor(
                                out=outp_sb[:nd, ob], in0=outp_sb[:nd, ob],
                                in1=B_sb[:nd, ob], op=mybir.AluOpType.add)
                            nc.sync.dma_start(out=outp_loc[b * 128:b * 128 + nd, :],
                                              in_=outp_sb[:nd, ob])
                        else:
                            res = evp.tile([128, 64], DT, tag="res")
                            nc.vector.tensor_scalar(
                                out=res[:nd], in0=tmp[:nd],
                                scalar1=C_sb[:nd, b:b + 1], scalar2=None,
                                op0=mybir.AluOpType.mult)
                            nc.vector.tensor_tensor(
                                out=res[:nd], in0=res[:nd],
                                in1=D_sb[:nd, ob], op=mybir.AluOpType.add)
                            nc.sync.dma_start(out=out_d[b * 128:b * 128 + nd, :],
                                              in_=res[:nd])
    nc.finalize()
    return nc


def kernel(x, edge_index, W1, b1, W2, b2):
    from concourse.bass_utils import run_bass_kernel_spmd

    in_maps, calls, T_total = _preprocess(x, edge_index, W1, b1, W2, b2)
    nc = _build(calls, T_total)
    trace = bool(int(os.environ.get("KERNEL_TRACE", "0")))
    if trace:
        import types
        mod = types.ModuleType("antenv.axon_hooks")
        mod._HOOK = None
        def _s(h): mod._HOOK = h
        def _g(): return mod._HOOK
        mod.set_axon_ntff_profile_hook = _s
        mod.get_axon_ntff_profile_hook = _g
        sys.modules["antenv.axon_hooks"] = mod
        import antenv
        antenv.axon_hooks = mod
        from trn_agent_boot.trn_boot import _ntff_profile_via_ctypes
        _s(_ntff_profile_via_ctypes('/opt/axon/libaxon_pjrt.so'))
        import concourse.bass_utils as bu
        bu.upload_artifacts = lambda tmpdir: "local://" + tmpdir
    res = run_bass_kernel_spmd(nc, in_maps, list(range(NCORES)), trace=trace)
    if trace and res.exec_time_ns:
        print(f"HW exec time: {res.exec_time_ns} ns")
    out = np.concatenate([res.results[c]["out"] for c in range(NCORES)], axis=0)
    return out


# revision 4
# speedup vs baseline: 1.4599x; 1.4599x over previous
"""APPNP GNN kernel for 8 Trainium2 NeuronCores.

Sharding: nodes (dst side) split into 8 contiguous shards of 12500.
Per step: all-gather of the dinv-scaled feature table (25.6 MB), then each
core gathers per-edge source rows from its local copy via dma_gather and
segment-sums them by dst with one-hot matmuls accumulating in PSUM.
Norms are folded into per-node scales: norm(s,d) = dinv[s]*dinv[d], so the
gathered table is pre-scaled by dinv and the dst-side dinv folds into the
PSUM evacuation. Self-loops are handled analytically (no gather).
"""
import os
import sys

sys.path.insert(0, "/opt/trn_rl_repo")

import numpy as np

N = 100000
E = 3200000
IN = 256
HID = 256
OUT = 64
K = 3
ALPHA = 0.1
NCORES = 8
NLOC = N // NCORES          # 12500
NBLK = (NLOC + 127) // 128  # 98 dst blocks per core
RANGE = 32768               # int16 index range
NRANGES = (N + RANGE - 1) // RANGE  # 4
MAX_TILES_PER_CALL = 8      # 1024 idxs per dma_gather (single-packet limit)


def _pack_idx_call(idx):
    """[n] int array -> [128, n//16] int16 tile (i -> [i%16, i//16], x8 replicated)."""
    n = len(idx)
    t = idx.reshape(n // 16, 16).T.astype(np.int16)
    return np.tile(t, (8, 1))


def _preprocess(x, edge_index, W1, b1, W2, b2):
    src = np.asarray(edge_index[0], dtype=np.int64)
    dst = np.asarray(edge_index[1], dtype=np.int64)
    deg = np.bincount(dst, minlength=N).astype(np.float64) + 1.0  # +self loop
    dinv = (1.0 / np.sqrt(deg)).astype(np.float32)

    # per-core edge buckets
    core_of = dst // NLOC
    per_core = []
    for c in range(NCORES):
        m = core_of == c
        s_c = src[m]
        d_c = dst[m] - c * NLOC
        blk = d_c // 128
        drel = d_c % 128
        rng_ = s_c // RANGE
        order = np.lexsort((s_c, rng_, blk))
        per_core.append((s_c[order], drel[order], blk[order], rng_[order]))

    # bucket counts per (core, block, range)
    cnt = np.zeros((NCORES, NBLK, NRANGES), dtype=np.int64)
    for c in range(NCORES):
        s_c, drel, blk, rng_ = per_core[c]
        np.add.at(cnt[c], (blk, rng_), 1)
    tiles_common = np.maximum(np.ceil(cnt / 128.0).astype(np.int64).max(axis=0), 0)

    # shared call schedule: list of (block, range, ntiles_in_call)
    calls = []
    for b in range(NBLK):
        for r in range(NRANGES):
            t = int(tiles_common[b, r])
            while t > 0:
                ct = min(t, MAX_TILES_PER_CALL)
                calls.append((b, r, ct))
                t -= ct
    T_total = int(tiles_common.sum())

    # per-core streams in the common layout
    idx_streams, dstrel_streams = [], []
    for c in range(NCORES):
        s_c, drel, blk, rng_ = per_core[c]
        # bucket start offsets in the sorted arrays
        starts = np.zeros((NBLK, NRANGES), dtype=np.int64)
        flat = cnt[c].reshape(-1)
        starts.reshape(-1)[1:] = np.cumsum(flat)[:-1]
        idx_cols = []    # per call: [128, ct*8] int16
        drel_cols = []   # per tile: [128] float32
        for b, r, ct in calls:
            # consume up to ct*128 edges from this bucket (tracked via starts/cnt copy)
            pass
        # do a second pass with mutable cursors
        cur = starts.copy()
        remaining = cnt[c].copy()
        for b, r, ct in calls:
            n_take = min(int(remaining[b, r]), ct * 128)
            off = int(cur[b, r])
            e_idx = (s_c[off:off + n_take] - r * RANGE).astype(np.int64)
            e_drel = drel[off:off + n_take].astype(np.float32)
            cur[b, r] += n_take
            remaining[b, r] -= n_take
            pad = ct * 128 - n_take
            if pad:
                e_idx = np.concatenate([e_idx, np.zeros(pad, np.int64)])
                e_drel = np.concatenate([e_drel, np.full(pad, -1.0, np.float32)])
            idx_cols.append(_pack_idx_call(e_idx))
            for j in range(ct):
                drel_cols.append(e_drel[j * 128:(j + 1) * 128])
        idx_streams.append(np.concatenate(idx_cols, axis=1))          # [128, 8*T_total]
        dstrel_streams.append(np.stack(drel_cols, axis=1))            # [128, T_total]

    # per-core dense inputs
    in_maps = []
    for c in range(NCORES):
        lo, hi = c * NLOC, (c + 1) * NLOC
        xT = np.ascontiguousarray(x[lo:hi].T)            # [256, 12500]
        dv = dinv[lo:hi]
        dinv_col = np.zeros((128, NBLK), np.float32)
        for b in range(NBLK):
            nblk = min(128, NLOC - b * 128)
            dinv_col[:nblk, b] = dv[b * 128:b * 128 + nblk]
        W1p = np.zeros((128, 512), np.float32)           # [k, (k*2+m) blocks]
        for k in range(2):
            for mblk in range(2):
                W1p[:, (k * 2 + mblk) * 128:(k * 2 + mblk + 1) * 128] = \
                    W1[k * 128:(k + 1) * 128, mblk * 128:(mblk + 1) * 128]
        W2p = np.zeros((128, 128), np.float32)           # [hid, m*64+o]
        for mblk in range(2):
            W2p[:, mblk * 64:(mblk + 1) * 64] = W2[mblk * 128:(mblk + 1) * 128, :]
        b1c = np.stack([b1[:128], b1[128:]], axis=1).astype(np.float32)   # [128, 2]
        b2r = np.tile(b2[None, :], (128, 1)).astype(np.float32)           # [128, 64]
        in_maps.append({
            "xT": xT.astype(np.float32),
            "idxs": idx_streams[c],
            "dstrel": dstrel_streams[c],
            "dinvc": dinv_col,
            "W1p": W1p, "W2p": W2p, "b1c": b1c, "b2r": b2r,
        })
    return in_maps, calls, T_total


def _build(calls, T_total):
    from concourse import bass, bacc, mybir
    from concourse.tile import TileContext

    DT = mybir.dt.float32
    nc = bacc.Bacc("TRN2", target_bir_lowering=False, debug=False,
                   num_devices=NCORES, num_swdge_queues=4)

    xT_d = nc.dram_tensor("xT", [IN, NLOC], DT, kind="ExternalInput").ap()
    idxs_d = nc.dram_tensor("idxs", [128, 8 * T_total], mybir.dt.int16,
                            kind="ExternalInput").ap()
    dstrel_d = nc.dram_tensor("dstrel", [128, T_total], DT,
                              kind="ExternalInput").ap()
    dinvc_d = nc.dram_tensor("dinvc", [128, NBLK], DT, kind="ExternalInput").ap()
    W1p_d = nc.dram_tensor("W1p", [128, 512], DT, kind="ExternalInput").ap()
    W2p_d = nc.dram_tensor("W2p", [128, 128], DT, kind="ExternalInput").ap()
    b1c_d = nc.dram_tensor("b1c", [128, 2], DT, kind="ExternalInput").ap()
    b2r_d = nc.dram_tensor("b2r", [128, 64], DT, kind="ExternalInput").ap()
    out_d = nc.dram_tensor("out", [NLOC, OUT], DT, kind="ExternalOutput").ap()

    outp_loc = nc.dram_tensor("outp_loc", [NLOC, OUT], DT).ap()
    tables = [nc.dram_tensor(f"table{k}", [N, OUT], DT, addr_space="Shared").ap()
              for k in range(K)]

    from concourse.masks import make_identity

    with TileContext(nc) as tc:
        with tc.tile_pool(name="const", bufs=1) as constp, \
             tc.tile_pool(name="persist", bufs=1) as persist:
            ident = constp.tile([128, 128], DT)
            make_identity(nc, ident[:])
            iota_f = constp.tile([128, MAX_TILES_PER_CALL * 128], DT)
            iota_i = constp.tile([128, MAX_TILES_PER_CALL * 128], mybir.dt.int32)
            nc.gpsimd.iota(iota_i[:], pattern=[[0, MAX_TILES_PER_CALL], [1, 128]],
                           base=0, channel_multiplier=0)
            nc.vector.tensor_copy(iota_f[:], iota_i[:])
            dinv_sb = constp.tile([128, NBLK], DT)
            nc.sync.dma_start(out=dinv_sb[:], in_=dinvc_d[:])
            A_sb = constp.tile([128, NBLK], DT)      # 0.9*dinv^2
            nc.vector.tensor_tensor(out=A_sb[:], in0=dinv_sb[:], in1=dinv_sb[:],
                                    op=mybir.AluOpType.mult)
            nc.vector.tensor_scalar_mul(A_sb[:], A_sb[:], 1.0 - ALPHA)
            C_sb = constp.tile([128, NBLK], DT)      # 0.9*dinv
            nc.vector.tensor_scalar_mul(C_sb[:], dinv_sb[:], 1.0 - ALPHA)
            dstrel_sb = persist.tile([128, T_total], DT)
            nc.sync.dma_start(out=dstrel_sb[:], in_=dstrel_d[:])
            B_sb = persist.tile([128, NBLK * OUT], DT)   # 0.1*dinv*h
            D_sb = persist.tile([128, NBLK * OUT], DT)   # 0.1*h
            outp_sb = persist.tile([128, NBLK * OUT], DT)  # dinv*out_prev (local)

            # ---------------- MLP ----------------
            with tc.tile_pool(name="mlpw", bufs=1) as mlpw, \
                 tc.tile_pool(name="mlp", bufs=3) as mlp, \
                 tc.tile_pool(name="mpsum", bufs=2, space="PSUM") as mpsum, \
                 tc.tile_pool(name="mpsum2", bufs=2, space="PSUM") as mpsum2:
                W1_sb = mlpw.tile([128, 512], DT)
                nc.sync.dma_start(out=W1_sb[:], in_=W1p_d[:])
                W2_sb = mlpw.tile([128, 128], DT)
                nc.sync.dma_start(out=W2_sb[:], in_=W2p_d[:])
                b1_sb = mlpw.tile([128, 2], DT)
                nc.sync.dma_start(out=b1_sb[:], in_=b1c_d[:])
                b2_sb = mlpw.tile([128, 64], DT)
                nc.sync.dma_start(out=b2_sb[:], in_=b2r_d[:])

                for rb in range(NBLK):
                    r0 = rb * 128
                    nrow = min(128, NLOC - r0)
                    xt = mlp.tile([128, 2 * 128], DT, tag="xt")
                    for k in range(2):
                        nc.sync.dma_start(
                            out=xt[:, k * 128:k * 128 + nrow],
                            in_=xT_d[k * 128:(k + 1) * 128, r0:r0 + nrow])
                    h1 = mlp.tile([128, 2 * 128], DT, tag="h1")
                    for mblk in range(2):
                        p1 = mpsum.tile([128, 128], DT, tag="p1")
                        for k in range(2):
                            nc.tensor.matmul(
                                out=p1[:, :nrow],
                                lhsT=W1_sb[:, (k * 2 + mblk) * 128:(k * 2 + mblk + 1) * 128],
                                rhs=xt[:, k * 128:k * 128 + nrow],
                                start=(k == 0), stop=(k == 1))
                        nc.scalar.activation(
                            h1[:, mblk * 128:mblk * 128 + nrow], p1[:, :nrow],
                            mybir.ActivationFunctionType.Relu,
                            bias=b1_sb[:, mblk:mblk + 1])
                    p2 = mpsum2.tile([128, 64], DT, tag="p2")
                    for mblk in range(2):
                        nc.tensor.matmul(
                            out=p2[:nrow, :],
                            lhsT=h1[:, mblk * 128:mblk * 128 + nrow],
                            rhs=W2_sb[:, mblk * 64:(mblk + 1) * 64],
                            start=(mblk == 0), stop=(mblk == 1))
                    ht = mlp.tile([128, 64], DT, tag="ht")
                    nc.vector.tensor_tensor(out=ht[:nrow], in0=p2[:nrow],
                                            in1=b2_sb[:nrow],
                                            op=mybir.AluOpType.add)
                    ob = slice(rb * OUT, rb * OUT + OUT)
                    # outp_sb = dinv*h ; B = 0.1*dinv*h ; D = 0.1*h
                    nc.vector.tensor_scalar(
                        out=outp_sb[:nrow, ob], in0=ht[:nrow],
                        scalar1=dinv_sb[:nrow, rb:rb + 1], scalar2=None,
                        op0=mybir.AluOpType.mult)
                    nc.vector.tensor_scalar_mul(B_sb[:nrow, ob],
                                                outp_sb[:nrow, ob], ALPHA)
                    nc.vector.tensor_scalar_mul(D_sb[:nrow, ob], ht[:nrow], ALPHA)
                    nc.sync.dma_start(out=outp_loc[r0:r0 + nrow, :],
                                      in_=outp_sb[:nrow, ob])

            # ---------------- propagation ----------------
            range_lens = [min(RANGE, N - r * RANGE) for r in range(NRANGES)]
            for step in range(K):
                table = tables[step]
                nc.gpsimd.collective_compute(
                    "AllGather", mybir.AluOpType.bypass,
                    replica_groups=[list(range(NCORES))],
                    ins=[outp_loc[:].opt()],
                    outs=[table[:].opt()])
                with tc.tile_pool(name=f"gat{step}", bufs=4) as gat, \
                     tc.tile_pool(name=f"idx{step}", bufs=4) as idxp, \
                     tc.tile_pool(name=f"sbl{step}", bufs=4) as sbl, \
                     tc.tile_pool(name=f"agg{step}", bufs=2) as aggp, \
                     tc.tile_pool(name=f"ev{step}", bufs=3) as evp, \
                     tc.tile_pool(name=f"ps{step}", bufs=2, space="PSUM") as psp, \
                     tc.tile_pool(name=f"pt{step}", bufs=2, space="PSUM") as ptp:
                    tile_cursor = 0
                    call_cursor = 0
                    cur_block = -1
                    aggT_ps = None
                    # group calls by block
                    from itertools import groupby
                    ci = 0
                    for b, bcalls_iter in groupby(calls, key=lambda x: x[0]):
                        bcalls = list(bcalls_iter)
                        ntiles_b = sum(ct for (_, _, ct) in bcalls)
                        nd = min(128, NLOC - b * 128)
                        aggT_ps = psp.tile([64, 128], DT, tag="aggT")
                        t_in_b = 0
                        for (_, r, ct) in bcalls:
                            it = idxp.tile([128, MAX_TILES_PER_CALL * 8],
                                           mybir.dt.int16, tag="it")
                            nc.sync.dma_start(
                                out=it[:, :ct * 8],
                                in_=idxs_d[:, tile_cursor * 8:(tile_cursor + ct) * 8])
                            gt = gat.tile([128, MAX_TILES_PER_CALL, OUT], DT, tag="gt")
                            nc.gpsimd.dma_gather(
                                gt[:, :ct, :],
                                table[r * RANGE:r * RANGE + range_lens[r]],
                                it[:, :ct * 8],
                                ct * 128, ct * 128, OUT,
                                single_packet=True,
                                queue_num=call_cursor % 4)
                            call_cursor += 1
                            st = sbl.tile([128, MAX_TILES_PER_CALL * 128], DT, tag="st")
                            nc.vector.tensor_tensor(
                                out=st[:, :ct * 128].rearrange(
                                    "p (a b) -> p a b", b=128),
                                in0=dstrel_sb[:, tile_cursor:tile_cursor + ct]
                                    .unsqueeze(2).to_broadcast([128, ct, 128]),
                                in1=iota_f[:, :ct * 128].rearrange(
                                    "p (a b) -> p a b", b=128),
                                op=mybir.AluOpType.is_equal)
                            for j in range(ct):
                                nc.tensor.matmul(
                                    out=aggT_ps[:, :],
                                    lhsT=gt[:, j, :],
                                    rhs=st[:, j * 128:(j + 1) * 128],
                                    start=(t_in_b == 0),
                                    stop=(t_in_b == ntiles_b - 1))
                                t_in_b += 1
                            tile_cursor += ct
                        # evacuate block b
                        aggT_sb = evp.tile([64, 128], DT, tag="aggTs")
                        nc.scalar.mul(aggT_sb[:], aggT_ps[:], 1.0)
                        tr_ps = ptp.tile([128, 64], DT, tag="tr")
                        nc.tensor.transpose(out=tr_ps[:, :], in_=aggT_sb[:, :],
                                            identity=ident[:64, :64])
                        ob = slice(b * OUT, b * OUT + OUT)
                        tmp = evp.tile([128, 64], DT, tag="tmp")
                        nc.vector.tensor_tensor(out=tmp[:nd], in0=tr_ps[:nd],
                                                in1=outp_sb[:nd, ob],
                                                op=mybir.AluOpType.add)
                        if step < K - 1:
                            nc.vector.tensor_scalar(
                                out=outp_sb[:nd, ob], in0=tmp[:nd],
                                scalar1=A_sb[:nd, b:b + 1], scalar2=None,
                                op0=mybir.AluOpType.mult)
                            nc.vector.tensor_tensor(
                                out=outp_sb[:nd, ob], in0=outp_sb[:nd, ob],
                                in1=B_sb[:nd, ob], op=mybir.AluOpType.add)
                            nc.sync.dma_start(out=outp_loc[b * 128:b * 128 + nd, :],
                                              in_=outp_sb[:nd, ob])
                        else:
                            res = evp.tile([128, 64], DT, tag="res")
                            nc.vector.tensor_scalar(
                                out=res[:nd], in0=tmp[:nd],
                                scalar1=C_sb[:nd, b:b + 1], scalar2=None,
                                op0=mybir.AluOpType.mult)
                            nc.vector.tensor_tensor(
                                out=res[:nd], in0=res[:nd],
                                in1=D_sb[:nd, ob], op=mybir.AluOpType.add)
                            nc.sync.dma_start(out=out_d[b * 128:b * 128 + nd, :],
                                              in_=res[:nd])
    nc.finalize()
    return nc


def kernel(x, edge_index, W1, b1, W2, b2):
    from concourse.bass_utils import run_bass_kernel_spmd

    in_maps, calls, T_total = _preprocess(x, edge_index, W1, b1, W2, b2)
    nc = _build(calls, T_total)
    trace = bool(int(os.environ.get("KERNEL_TRACE", "0")))
    if trace:
        import types
        mod = types.ModuleType("antenv.axon_hooks")
        mod._HOOK = None
        def _s(h): mod._HOOK = h
        def _g(): return mod._HOOK
        mod.set_axon_ntff_profile_hook = _s
        mod.get_axon_ntff_profile_hook = _g
        sys.modules["antenv.axon_hooks"] = mod
        import antenv
        antenv.axon_hooks = mod
        from trn_agent_boot.trn_boot import _ntff_profile_via_ctypes
        _s(_ntff_profile_via_ctypes('/opt/axon/libaxon_pjrt.so'))
        import concourse.bass_utils as bu
        bu.upload_artifacts = lambda tmpdir: "local://" + tmpdir
    res = run_bass_kernel_spmd(nc, in_maps, list(range(NCORES)), trace=trace)
    if trace and res.exec_time_ns:
        print(f"HW exec time: {res.exec_time_ns} ns")
    out = np.concatenate([res.results[c]["out"] for c in range(NCORES)], axis=0)
    return out


# revision 5
# speedup vs baseline: 1.8880x; 1.2932x over previous
"""APPNP GNN kernel for 8 Trainium2 NeuronCores.

Sharding: nodes (dst side) split into 8 contiguous shards of 12500.
Per step: all-gather of the dinv-scaled feature table (25.6 MB), then each
core gathers per-edge source rows from its local copy via dma_gather and
segment-sums them by dst with one-hot matmuls accumulating in PSUM.
Norms are folded into per-node scales: norm(s,d) = dinv[s]*dinv[d], so the
gathered table is pre-scaled by dinv and the dst-side dinv folds into the
PSUM evacuation. Self-loops are handled analytically (no gather).
"""
import os
import sys

sys.path.insert(0, "/opt/trn_rl_repo")

import numpy as np

N = 100000
E = 3200000
IN = 256
HID = 256
OUT = 64
K = 3
ALPHA = 0.1
NCORES = 8
NLOC = N // NCORES          # 12500
NBLK = (NLOC + 127) // 128  # 98 dst blocks per core
RANGE = 32768               # int16 index range
NRANGES = (N + RANGE - 1) // RANGE  # 4
MAX_TILES_PER_CALL = 8      # 1024 idxs per dma_gather (single-packet limit)


def _pack_idx_call(idx):
    """[n] int array -> [128, n//16] int16 tile (i -> [i%16, i//16], x8 replicated)."""
    n = len(idx)
    t = idx.reshape(n // 16, 16).T.astype(np.int16)
    return np.tile(t, (8, 1))


def _preprocess(x, edge_index, W1, b1, W2, b2):
    src = np.asarray(edge_index[0], dtype=np.int64)
    dst = np.asarray(edge_index[1], dtype=np.int64)
    deg = np.bincount(dst, minlength=N).astype(np.float64) + 1.0  # +self loop
    dinv = (1.0 / np.sqrt(deg)).astype(np.float32)

    # per-core edge buckets
    core_of = dst // NLOC
    per_core = []
    for c in range(NCORES):
        m = core_of == c
        s_c = src[m]
        d_c = dst[m] - c * NLOC
        blk = d_c // 128
        drel = d_c % 128
        rng_ = s_c // RANGE
        order = np.lexsort((s_c, rng_, blk))
        per_core.append((s_c[order], drel[order], blk[order], rng_[order]))

    # bucket counts per (core, block, range)
    cnt = np.zeros((NCORES, NBLK, NRANGES), dtype=np.int64)
    for c in range(NCORES):
        s_c, drel, blk, rng_ = per_core[c]
        np.add.at(cnt[c], (blk, rng_), 1)
    tiles_common = np.maximum(np.ceil(cnt / 128.0).astype(np.int64).max(axis=0), 0)

    # shared call schedule: list of (block, range, ntiles_in_call)
    calls = []
    for b in range(NBLK):
        for r in range(NRANGES):
            t = int(tiles_common[b, r])
            while t > 0:
                ct = min(t, MAX_TILES_PER_CALL)
                calls.append((b, r, ct))
                t -= ct
    T_total = int(tiles_common.sum())

    # per-core streams in the common layout
    idx_streams, dstrel_streams = [], []
    for c in range(NCORES):
        s_c, drel, blk, rng_ = per_core[c]
        # bucket start offsets in the sorted arrays
        starts = np.zeros((NBLK, NRANGES), dtype=np.int64)
        flat = cnt[c].reshape(-1)
        starts.reshape(-1)[1:] = np.cumsum(flat)[:-1]
        idx_cols = []    # per call: [128, ct*8] int16
        drel_cols = []   # per tile: [128] float32
        for b, r, ct in calls:
            # consume up to ct*128 edges from this bucket (tracked via starts/cnt copy)
            pass
        # do a second pass with mutable cursors
        cur = starts.copy()
        remaining = cnt[c].copy()
        for b, r, ct in calls:
            n_take = min(int(remaining[b, r]), ct * 128)
            off = int(cur[b, r])
            e_idx = (s_c[off:off + n_take] - r * RANGE).astype(np.int64)
            e_drel = drel[off:off + n_take].astype(np.float32)
            cur[b, r] += n_take
            remaining[b, r] -= n_take
            pad = ct * 128 - n_take
            if pad:
                e_idx = np.concatenate([e_idx, np.zeros(pad, np.int64)])
                e_drel = np.concatenate([e_drel, np.full(pad, -1.0, np.float32)])
            idx_cols.append(_pack_idx_call(e_idx))
            for j in range(ct):
                drel_cols.append(e_drel[j * 128:(j + 1) * 128])
        idx_streams.append(np.concatenate(idx_cols, axis=1))          # [128, 8*T_total]
        dstrel_streams.append(np.stack(drel_cols, axis=1))            # [128, T_total]

    # per-core dense inputs
    in_maps = []
    for c in range(NCORES):
        lo, hi = c * NLOC, (c + 1) * NLOC
        xT = np.ascontiguousarray(x[lo:hi].T)            # [256, 12500]
        dv = dinv[lo:hi]
        dinv_col = np.zeros((128, NBLK), np.float32)
        for b in range(NBLK):
            nblk = min(128, NLOC - b * 128)
            dinv_col[:nblk, b] = dv[b * 128:b * 128 + nblk]
        W1p = np.zeros((128, 512), np.float32)           # [k, (k*2+m) blocks]
        for k in range(2):
            for mblk in range(2):
                W1p[:, (k * 2 + mblk) * 128:(k * 2 + mblk + 1) * 128] = \
                    W1[k * 128:(k + 1) * 128, mblk * 128:(mblk + 1) * 128]
        W2p = np.zeros((128, 128), np.float32)           # [hid, m*64+o]
        for mblk in range(2):
            W2p[:, mblk * 64:(mblk + 1) * 64] = W2[mblk * 128:(mblk + 1) * 128, :]
        b1c = np.stack([b1[:128], b1[128:]], axis=1).astype(np.float32)   # [128, 2]
        b2r = np.tile(b2[None, :], (128, 1)).astype(np.float32)           # [128, 64]
        in_maps.append({
            "xT": xT.astype(np.float32),
            "idxs": idx_streams[c],
            "dstrel": dstrel_streams[c],
            "dinvc": dinv_col,
            "W1p": W1p, "W2p": W2p, "b1c": b1c, "b2r": b2r,
        })
    return in_maps, calls, T_total


def _build(calls, T_total):
    from concourse import bass, bacc, mybir
    from concourse.tile import TileContext

    DT = mybir.dt.float32
    nc = bacc.Bacc("TRN2", target_bir_lowering=False, debug=False,
                   num_devices=NCORES, num_swdge_queues=4)

    xT_d = nc.dram_tensor("xT", [IN, NLOC], DT, kind="ExternalInput").ap()
    idxs_d = nc.dram_tensor("idxs", [128, 8 * T_total], mybir.dt.int16,
                            kind="ExternalInput").ap()
    dstrel_d = nc.dram_tensor("dstrel", [128, T_total], DT,
                              kind="ExternalInput").ap()
    dinvc_d = nc.dram_tensor("dinvc", [128, NBLK], DT, kind="ExternalInput").ap()
    W1p_d = nc.dram_tensor("W1p", [128, 512], DT, kind="ExternalInput").ap()
    W2p_d = nc.dram_tensor("W2p", [128, 128], DT, kind="ExternalInput").ap()
    b1c_d = nc.dram_tensor("b1c", [128, 2], DT, kind="ExternalInput").ap()
    b2r_d = nc.dram_tensor("b2r", [128, 64], DT, kind="ExternalInput").ap()
    out_d = nc.dram_tensor("out", [NLOC, OUT], DT, kind="ExternalOutput").ap()

    outp_loc = nc.dram_tensor("outp_loc", [NLOC, OUT], DT).ap()
    tables = [nc.dram_tensor(f"table{k}", [N, OUT], DT, addr_space="Shared").ap()
              for k in range(K)]

    from concourse.masks import make_identity

    with TileContext(nc) as tc:
        with tc.tile_pool(name="const", bufs=1) as constp, \
             tc.tile_pool(name="persist", bufs=1) as persist:
            ident = constp.tile([128, 128], DT)
            make_identity(nc, ident[:])
            iota_f = constp.tile([128, MAX_TILES_PER_CALL * 128], DT)
            iota_i = constp.tile([128, MAX_TILES_PER_CALL * 128], mybir.dt.int32)
            nc.gpsimd.iota(iota_i[:], pattern=[[0, MAX_TILES_PER_CALL], [1, 128]],
                           base=0, channel_multiplier=0)
            nc.vector.tensor_copy(iota_f[:], iota_i[:])
            dinv_sb = constp.tile([128, NBLK], DT)
            nc.sync.dma_start(out=dinv_sb[:], in_=dinvc_d[:])
            A_sb = constp.tile([128, NBLK], DT)      # 0.9*dinv^2
            nc.vector.tensor_tensor(out=A_sb[:], in0=dinv_sb[:], in1=dinv_sb[:],
                                    op=mybir.AluOpType.mult)
            nc.vector.tensor_scalar_mul(A_sb[:], A_sb[:], 1.0 - ALPHA)
            C_sb = constp.tile([128, NBLK], DT)      # 0.9*dinv
            nc.vector.tensor_scalar_mul(C_sb[:], dinv_sb[:], 1.0 - ALPHA)
            dstrel_sb = persist.tile([128, T_total], DT)
            nc.sync.dma_start(out=dstrel_sb[:], in_=dstrel_d[:])
            B_sb = persist.tile([128, NBLK * OUT], DT)   # 0.1*dinv*h
            D_sb = persist.tile([128, NBLK * OUT], DT)   # 0.1*h
            outp_sb = persist.tile([128, NBLK * OUT], DT)  # dinv*out_prev (local)

            # ---------------- MLP ----------------
            with tc.tile_pool(name="mlpw", bufs=1) as mlpw, \
                 tc.tile_pool(name="mlp", bufs=3) as mlp, \
                 tc.tile_pool(name="mpsum", bufs=2, space="PSUM") as mpsum, \
                 tc.tile_pool(name="mpsum2", bufs=2, space="PSUM") as mpsum2:
                W1_sb = mlpw.tile([128, 512], DT)
                nc.sync.dma_start(out=W1_sb[:], in_=W1p_d[:])
                W2_sb = mlpw.tile([128, 128], DT)
                nc.sync.dma_start(out=W2_sb[:], in_=W2p_d[:])
                b1_sb = mlpw.tile([128, 2], DT)
                nc.sync.dma_start(out=b1_sb[:], in_=b1c_d[:])
                b2_sb = mlpw.tile([128, 64], DT)
                nc.sync.dma_start(out=b2_sb[:], in_=b2r_d[:])

                for rb in range(NBLK):
                    r0 = rb * 128
                    nrow = min(128, NLOC - r0)
                    xt = mlp.tile([128, 2 * 128], DT, tag="xt")
                    for k in range(2):
                        nc.sync.dma_start(
                            out=xt[:, k * 128:k * 128 + nrow],
                            in_=xT_d[k * 128:(k + 1) * 128, r0:r0 + nrow])
                    h1 = mlp.tile([128, 2 * 128], DT, tag="h1")
                    for mblk in range(2):
                        p1 = mpsum.tile([128, 128], DT, tag="p1")
                        for k in range(2):
                            nc.tensor.matmul(
                                out=p1[:, :nrow],
                                lhsT=W1_sb[:, (k * 2 + mblk) * 128:(k * 2 + mblk + 1) * 128],
                                rhs=xt[:, k * 128:k * 128 + nrow],
                                start=(k == 0), stop=(k == 1))
                        nc.scalar.activation(
                            h1[:, mblk * 128:mblk * 128 + nrow], p1[:, :nrow],
                            mybir.ActivationFunctionType.Relu,
                            bias=b1_sb[:, mblk:mblk + 1])
                    p2 = mpsum2.tile([128, 64], DT, tag="p2")
                    for mblk in range(2):
                        nc.tensor.matmul(
                            out=p2[:nrow, :],
                            lhsT=h1[:, mblk * 128:mblk * 128 + nrow],
                            rhs=W2_sb[:, mblk * 64:(mblk + 1) * 64],
                            start=(mblk == 0), stop=(mblk == 1))
                    ht = mlp.tile([128, 64], DT, tag="ht")
                    nc.vector.tensor_tensor(out=ht[:nrow], in0=p2[:nrow],
                                            in1=b2_sb[:nrow],
                                            op=mybir.AluOpType.add)
                    ob = slice(rb * OUT, rb * OUT + OUT)
                    # outp_sb = dinv*h ; B = 0.1*dinv*h ; D = 0.1*h
                    nc.vector.tensor_scalar(
                        out=outp_sb[:nrow, ob], in0=ht[:nrow],
                        scalar1=dinv_sb[:nrow, rb:rb + 1], scalar2=None,
                        op0=mybir.AluOpType.mult)
                    nc.vector.tensor_scalar_mul(B_sb[:nrow, ob],
                                                outp_sb[:nrow, ob], ALPHA)
                    nc.vector.tensor_scalar_mul(D_sb[:nrow, ob], ht[:nrow], ALPHA)
                    nc.sync.dma_start(out=outp_loc[r0:r0 + nrow, :],
                                      in_=outp_sb[:nrow, ob])

            # ---------------- propagation ----------------
            range_lens = [min(RANGE, N - r * RANGE) for r in range(NRANGES)]
            for step in range(K):
                table = tables[step]
                nc.gpsimd.collective_compute(
                    "AllGather", mybir.AluOpType.bypass,
                    replica_groups=[list(range(NCORES))],
                    ins=[outp_loc[:].opt()],
                    outs=[table[:].opt()])
                with tc.tile_pool(name=f"gat{step}", bufs=12) as gat, \
                     tc.tile_pool(name=f"idx{step}", bufs=12) as idxp, \
                     tc.tile_pool(name=f"sbl{step}", bufs=8) as sbl, \
                     tc.tile_pool(name=f"agg{step}", bufs=2) as aggp, \
                     tc.tile_pool(name=f"ev{step}", bufs=3) as evp, \
                     tc.tile_pool(name=f"ps{step}", bufs=2, space="PSUM") as psp, \
                     tc.tile_pool(name=f"pt{step}", bufs=2, space="PSUM") as ptp:
                    tile_cursor = 0
                    call_cursor = 0
                    cur_block = -1
                    aggT_ps = None
                    # group calls by block
                    from itertools import groupby
                    ci = 0
                    for b, bcalls_iter in groupby(calls, key=lambda x: x[0]):
                        bcalls = list(bcalls_iter)
                        ntiles_b = sum(ct for (_, _, ct) in bcalls)
                        nd = min(128, NLOC - b * 128)
                        aggT_ps = psp.tile([64, 128], DT, tag="aggT")
                        t_in_b = 0
                        for (_, r, ct) in bcalls:
                            it = idxp.tile([128, MAX_TILES_PER_CALL * 8],
                                           mybir.dt.int16, tag="it")
                            nc.sync.dma_start(
                                out=it[:, :ct * 8],
                                in_=idxs_d[:, tile_cursor * 8:(tile_cursor + ct) * 8])
                            gt = gat.tile([128, MAX_TILES_PER_CALL, OUT], DT, tag="gt")
                            nc.gpsimd.dma_gather(
                                gt[:, :ct, :],
                                table[r * RANGE:r * RANGE + range_lens[r]],
                                it[:, :ct * 8],
                                ct * 128, ct * 128, OUT,
                                single_packet=True,
                                queue_num=call_cursor % 4)
                            call_cursor += 1
                            st = sbl.tile([128, MAX_TILES_PER_CALL * 128], DT, tag="st")
                            nc.vector.tensor_tensor(
                                out=st[:, :ct * 128].rearrange(
                                    "p (a b) -> p a b", b=128),
                                in0=dstrel_sb[:, tile_cursor:tile_cursor + ct]
                                    .unsqueeze(2).to_broadcast([128, ct, 128]),
                                in1=iota_f[:, :ct * 128].rearrange(
                                    "p (a b) -> p a b", b=128),
                                op=mybir.AluOpType.is_equal)
                            for j in range(ct):
                                nc.tensor.matmul(
                                    out=aggT_ps[:, :],
                                    lhsT=gt[:, j, :],
                                    rhs=st[:, j * 128:(j + 1) * 128],
                                    start=(t_in_b == 0),
                                    stop=(t_in_b == ntiles_b - 1))
                                t_in_b += 1
                            tile_cursor += ct
                        # evacuate block b
                        aggT_sb = evp.tile([64, 128], DT, tag="aggTs")
                        nc.scalar.mul(aggT_sb[:], aggT_ps[:], 1.0)
                        tr_ps = ptp.tile([128, 64], DT, tag="tr")
                        nc.tensor.transpose(out=tr_ps[:, :], in_=aggT_sb[:, :],
                                            identity=ident[:64, :64])
                        ob = slice(b * OUT, b * OUT + OUT)
                        tmp = evp.tile([128, 64], DT, tag="tmp")
                        nc.vector.tensor_tensor(out=tmp[:nd], in0=tr_ps[:nd],
                                                in1=outp_sb[:nd, ob],
                                                op=mybir.AluOpType.add)
                        if step < K - 1:
                            nc.vector.tensor_scalar(
                                out=outp_sb[:nd, ob], in0=tmp[:nd],
                                scalar1=A_sb[:nd, b:b + 1], scalar2=None,
                                op0=mybir.AluOpType.mult)
                            nc.vector.tensor_tensor(
                                out=outp_sb[:nd, ob], in0=outp_sb[:nd, ob],
                                in1=B_sb[:nd, ob], op=mybir.AluOpType.add)
                            nc.sync.dma_start(out=outp_loc[b * 128:b * 128 + nd, :],
                                              in_=outp_sb[:nd, ob])
                        else:
                            res = evp.tile([128, 64], DT, tag="res")
                            nc.vector.tensor_scalar(
                                out=res[:nd], in0=tmp[:nd],
                                scalar1=C_sb[:nd, b:b + 1], scalar2=None,
                                op0=mybir.AluOpType.mult)
                            nc.vector.tensor_tensor(
                                out=res[:nd], in0=res[:nd],
                                in1=D_sb[:nd, ob], op=mybir.AluOpType.add)
                            nc.sync.dma_start(out=out_d[b * 128:b * 128 + nd, :],
                                              in_=res[:nd])
    nc.finalize()
    return nc


def kernel(x, edge_index, W1, b1, W2, b2):
    from concourse.bass_utils import run_bass_kernel_spmd

    in_maps, calls, T_total = _preprocess(x, edge_index, W1, b1, W2, b2)
    nc = _build(calls, T_total)
    trace = bool(int(os.environ.get("KERNEL_TRACE", "0")))
    if trace:
        import types
        mod = types.ModuleType("antenv.axon_hooks")
        mod._HOOK = None
        def _s(h): mod._HOOK = h
        def _g(): return mod._HOOK
        mod.set_axon_ntff_profile_hook = _s
        mod.get_axon_ntff_profile_hook = _g
        sys.modules["antenv.axon_hooks"] = mod
        import antenv
        antenv.axon_hooks = mod
        from trn_agent_boot.trn_boot import _ntff_profile_via_ctypes
        _s(_ntff_profile_via_ctypes('/opt/axon/libaxon_pjrt.so'))
        import concourse.bass_utils as bu
        bu.upload_artifacts = lambda tmpdir: "local://" + tmpdir
    res = run_bass_kernel_spmd(nc, in_maps, list(range(NCORES)), trace=trace)
    if trace and res.exec_time_ns:
        print(f"HW exec time: {res.exec_time_ns} ns")
    out = np.concatenate([res.results[c]["out"] for c in range(NCORES)], axis=0)
    return out
